# revision 4
# baseline (speedup 1.0000x reference)
"""DPQ forward (vq_codebook) Trainium2 Bass kernel.

Problem: x (65536, 512) f32, codebook (8, 256, 64) f32.
  xm = x.reshape(n, 8, 64)
  d2[n,m,k] = ||xm[n,m] - codebook[m,k]||^2
  codes = argmin_k d2                      -> (n, 8) int32
  recon[n,m,:] = codebook[m, codes[n,m]]
  x_recon = recon.reshape(n, 512)
  side_output = recon.transpose(1, 0, 2)   -> (8, n, 64)

Strategy (data-parallel over n across 8 cores, 8192 rows each):
  - argmin_k d2 == argmax_k (x.c_k - |c_k|^2/2). Computed per 128-row tile as
    8 fp32 matmuls (K=65 contraction: 64 dims + a ones-row folding the bias)
    into PSUM (128, 8*256).
  - VectorE: chunked tensor_reduce(max) -> per-(row,m) max; max_index over the
    2048-wide PSUM row -> flat position m*256+k.
  - Positions shuffled into dma_gather's wrapped int16 index layout via small
    SBUF->SBUF DMAs; one dma_gather per 8-tile group fetches the selected
    codewords (256 B each) from HBM into (128, 64, 64) recon tiles.
  - Stores: x_recon / codes / side_output via strided DMAs.

Host side does layout-only prep: transpose of x (+ones rows), codebook
reshape/transpose, |c|^2/2 rows, and the (row,m)->m*256 constant.
"""
import numpy as np

import concourse.bacc as bacc
import concourse.mybir as mybir
import concourse.tile as tile
from concourse.bass_utils import run_bass_kernel_spmd

N, D, M, K, DS = 65536, 512, 8, 256, 64
NC = 8                      # cores
NL = N // NC                # rows per core = 8192
P = 128                     # rows per tile
NT = NL // P                # tiles per core = 64
TG = 8                      # tiles per group
NG = NT // TG               # groups per core = 8
ROWS_G = P * TG             # rows per group = 1024
FREE = M * K                # 2048
NIDX = P * TG * M           # gather indices per group = 8192

_cache = {}


def _build():
    nc = bacc.Bacc("TRN2", target_bir_lowering=False, debug=False, num_devices=NC)

    # inputs (per core)
    xt_d = nc.declare_dram_parameter("xt", [M * (DS + 1), NL], mybir.dt.float32, isOutput=False)
    cbt_d = nc.declare_dram_parameter("cbt", [DS + 1, FREE], mybir.dt.float32, isOutput=False)
    cbf_d = nc.declare_dram_parameter("cbf", [M * K, DS], mybir.dt.float32, isOutput=False)
    msub_d = nc.declare_dram_parameter("msub", [P, TG * M], mybir.dt.int32, isOutput=False)

    # outputs (per core)
    xr_d = nc.declare_dram_parameter("xr", [NL, D], mybir.dt.float32, isOutput=True)
    codes_d = nc.declare_dram_parameter("codes", [NL, M], mybir.dt.int32, isOutput=True)
    side_d = nc.declare_dram_parameter("side", [M * NL, DS], mybir.dt.float32, isOutput=True)

    with tile.TileContext(nc) as tc:
        with tc.tile_pool(name="const", bufs=1) as cpool, \
             tc.tile_pool(name="slab", bufs=16) as spool, \
             tc.tile_pool(name="ps", bufs=2, space="PSUM") as ppool, \
             tc.tile_pool(name="small", bufs=3) as mpool, \
             tc.tile_pool(name="recon", bufs=2) as rpool:

            cbt = cpool.tile([DS + 1, FREE], mybir.dt.float32, tag="cbt")
            nc.sync.dma_start(cbt[:], cbt_d[:])
            msub = cpool.tile([P, TG * M], mybir.dt.int32, tag="msub")
            nc.sync.dma_start(msub[:], msub_d[:])

            for g in range(NG):
                # ---- load xT slabs for this group: (65, 1024) per m ----
                slabs = []
                for m in range(M):
                    sl = spool.tile([DS + 1, ROWS_G], mybir.dt.float32, tag="slab")
                    nc.sync.dma_start(
                        sl[:], xt_d[m * (DS + 1):(m + 1) * (DS + 1),
                                    g * ROWS_G:(g + 1) * ROWS_G])
                    slabs.append(sl)

                pos = mpool.tile([P, TG * M], mybir.dt.uint16, tag="pos")

                for t in range(TG):
                    sc = ppool.tile([P, FREE], mybir.dt.float32, tag="sc")
                    for m in range(M):
                        nc.tensor.matmul(
                            sc[:, m * K:(m + 1) * K],
                            lhsT=slabs[m][:, t * P:(t + 1) * P],
                            rhs=cbt[:, m * K:(m + 1) * K],
                            start=True, stop=True)
                    maxv = mpool.tile([P, M], mybir.dt.float32, tag="maxv")
                    nc.vector.tensor_reduce(
                        maxv[:], sc[:].rearrange("p (m k) -> p m k", m=M),
                        axis=mybir.AxisListType.X, op=mybir.AluOpType.max)
                    nc.vector.max_index(pos[:, t * M:(t + 1) * M], maxv[:], sc[:])

                # ---- codes = pos - m*256, int32, stored as (p, t, m) ----
                pos_i32 = mpool.tile([P, TG * M], mybir.dt.int32, tag="pos32")
                nc.vector.tensor_copy(pos_i32[:], pos[:])
                codes_g = mpool.tile([P, TG * M], mybir.dt.int32, tag="codesg")
                nc.vector.tensor_tensor(
                    out=codes_g[:], in0=pos_i32[:], in1=msub[:],
                    op=mybir.AluOpType.subtract)
                nc.sync.dma_start(
                    codes_d[g * ROWS_G:(g + 1) * ROWS_G, :]
                        .rearrange("(t p) m -> p t m", p=P),
                    codes_g[:].rearrange("p (t m) -> p t m", t=TG))

                # ---- wrapped gather indices (128, 512) i16 ----
                idxs = mpool.tile([P, NIDX // 16], mybir.dt.int16, tag="idxs")
                pos_i16 = pos[:].bitcast(mybir.dt.int16)
                for s in range(8):
                    nc.sync.dma_start(
                        idxs[0:16, :].rearrange("q (c s) -> q c s", s=8)[:, :, s:s + 1],
                        pos_i16[s * 16:(s + 1) * 16, :]
                            .rearrange("q (c one) -> q c one", one=1))
                for r in range(1, 8):
                    nc.sync.dma_start(idxs[16 * r:16 * (r + 1), :], idxs[0:16, :])

                # ---- gather codewords: recon (128, 64 groups, 64) ----
                # dma_gather tops out between 1024 and 2048 indices per call
                # (SWDGE ring), so issue one 1024-index gather per tile.
                recon = rpool.tile([P, TG * M * DS], mybir.dt.float32, tag="recon")
                for t in range(TG):
                    nc.gpsimd.dma_gather(
                        out_ap=recon[:, t * M * DS:(t + 1) * M * DS]
                            .rearrange("p (g d) -> p g d", d=DS),
                        in_ap=cbf_d[:],
                        idxs_ap=idxs[:, t * (P * M // 16):(t + 1) * (P * M // 16)],
                        num_idxs=P * M,
                        num_idxs_reg=P * M,
                        elem_size=DS)

                # ---- stores ----
                nc.sync.dma_start(
                    xr_d[g * ROWS_G:(g + 1) * ROWS_G, :]
                        .rearrange("(t p) d -> p t d", p=P),
                    recon[:].rearrange("p (t d) -> p t d", t=TG))
                for m in range(M):
                    nc.sync.dma_start(
                        side_d[m * NL + g * ROWS_G: m * NL + (g + 1) * ROWS_G, :]
                            .rearrange("(t p) d -> p t d", p=P),
                        recon[:].rearrange("p (t m d) -> p t m d", t=TG, m=M)[:, :, m, :])

    nc.compile()
    return nc


def kernel(x, codebook):
    x = np.asarray(x, dtype=np.float32)
    codebook = np.asarray(codebook, dtype=np.float32)
    assert x.shape == (N, D) and codebook.shape == (M, K, DS)

    if "nc" not in _cache:
        _cache["nc"] = _build()
    nc = _cache["nc"]

    # ---- host-side layout prep ----
    # xt: (8, 65, N): per m, rows 0..63 = x[:, m*64:(m+1)*64].T, row 64 = 1.0
    xt = np.empty((M, DS + 1, N), dtype=np.float32)
    xt[:, :DS, :] = np.ascontiguousarray(
        x.reshape(N, M, DS).transpose(1, 2, 0))
    xt[:, DS, :] = 1.0
    xt = xt.reshape(M * (DS + 1), N)

    # cbt: (65, 2048): rows 0..63 = codebook[m,k,d] at col m*256+k; row 64 = -|c|^2/2
    cbt = np.empty((DS + 1, FREE), dtype=np.float32)
    cbt[:DS, :] = codebook.transpose(2, 0, 1).reshape(DS, FREE)
    cbt[DS, :] = -0.5 * (codebook.astype(np.float64) ** 2).sum(-1).reshape(FREE)

    cbf = np.ascontiguousarray(codebook.reshape(M * K, DS))

    msub = np.broadcast_to(
        (np.arange(TG * M, dtype=np.int32) % M) * K, (P, TG * M)).copy()

    in_maps = []
    for c in range(NC):
        in_maps.append(dict(
            xt=np.ascontiguousarray(xt[:, c * NL:(c + 1) * NL]),
            cbt=cbt, cbf=cbf, msub=msub))

    res = run_bass_kernel_spmd(nc, in_maps, core_ids=list(range(NC)))
    _cache["last_results"] = res

    x_recon = np.concatenate([r["xr"] for r in res.results], axis=0)
    codes = np.concatenate([r["codes"] for r in res.results], axis=0)
    side = np.concatenate(
        [r["side"].reshape(M, NL, DS) for r in res.results], axis=1)
    return x_recon, codes, side
